# revision 1
# baseline (speedup 1.0000x reference)
"""Trainium2 Bass kernel for the NeuralODE Euler-scan problem (v2).

Math reformulation (per core, local batch BL=512 split into 2 blocks of 256):
  reference: x_{t+1} = x_t + dt*(tanh([x_t, I_t] @ W1 + b1) @ W2 + b2)
  we track the pre-activation y_t = x_t @ W1x + I_t*w1i + b1 resident in PSUM:
      h_t     = tanh(y_t)                               (ACT, psum -> sbuf)
      y_{t+1} = y_t + h_t @ (dt*W2@W1x) + dI_t*w1i + dt*b2@W1x   (PE, accum)
      delta_t = h_t @ (dt*W2)                           (PE -> psum, evac DVE)
  x_t is reconstructed on the host: x_t = x0 + cumsum(delta + dt*b2).

v2 change vs v1: the 256-sample free dim is split into TWO staggered streams
(A = cols 0:128, B = cols 128:256), each with its own pair of parity PSUM
banks.  The serial chain per stream step is tanh -> window matmul -> tanh
(~545ns incl. 2 sem hops), but while stream A's matmul+sems complete the ACT
engine runs stream B's tanh, so the steady-state period is set by ACT
throughput: 2 x (128*0.83 + 185)ns = 584ns per step instead of the chain-bound
705ns of the unsplit kernel.  PSUM y state is 4 separate tiles (stream x
parity) because PSUM dependency tracking is bank-granular: sharing a tile
between streams creates false cross-stream deps that re-serialize the chain.

Layout: transposed + block-diagonal over the 2 batch blocks, so y/h live as
[128 partitions = 2 blocks x 64 hidden, 128 samples] per stream.  All
recurrence matmuls run in float16 (full PE rate at any moving-dim size,
unlike float32r which needs N>=256; 10-bit mantissa keeps the accumulated
error ~2e-3, well inside the 2e-2 gate).

Output: per-step-pair deltas land in [32, 512] psum tiles, are packed by
partition-shifted DVE copies into a [128, 512] stage tile (8 steps) and
DMA'd out raw; the host decodes, adds dt*b2, and cumsums.  The batch dim
(4096) is sharded across the 8 cores; each runs this same program.

di prefetch DMA uses a host-transposed (8, nmmi, S) layout so each group
load is 8 contiguous descriptors instead of 240 strided ones.
"""

import os
import numpy as np

import concourse.bass as bass
from concourse import bacc
import concourse.mybir as mybir
from concourse.tile import TileContext
from concourse import bass_utils

B, T, D, H = 4096, 512, 16, 64
NCORES = 8
BL = B // NCORES          # 512 samples per core
S = BL // 2               # 256 samples per block
HALF = S // 2             # 128 samples per stream
NSTEP = T - 1             # 511 Euler steps
GPF = 30                  # dI prefetch group size (510 = 17*30)

f32 = mybir.dt.float32
f32r = mybir.dt.float32r
f16 = mybir.dt.float16
TANH = mybir.ActivationFunctionType.Tanh


def build_nc(nstep=NSTEP, nchunk=None):
    nmmi = nstep - 1                # number of y-update steps (di8 rows)
    if nchunk is None:
        nchunk = (nstep + 7) // 8
    nc = bacc.Bacc("TRN2", target_bir_lowering=False, debug=False)

    wzz_d = nc.dram_tensor("wzz", (128, 128), f16, kind="ExternalInput")
    w1i_d = nc.dram_tensor("w1i", (8, 128), f16, kind="ExternalInput")
    w2d_d = nc.dram_tensor("w2d", (128, 32), f16, kind="ExternalInput")
    w1x_d = nc.dram_tensor("w1x", (32, 128), f16, kind="ExternalInput")
    ib_d = nc.dram_tensor("ib", (4, 128), f16, kind="ExternalInput")
    x0t_d = nc.dram_tensor("x0t", (32, S), f16, kind="ExternalInput")
    i0b_d = nc.dram_tensor("i0b", (4, S), f16, kind="ExternalInput")
    di_d = nc.dram_tensor("di", (8, max(nmmi, 1), S), f16, kind="ExternalInput")
    out_d = nc.dram_tensor("delta", (nchunk, 128, 512), f32, kind="ExternalOutput")

    with TileContext(nc) as tc:
        with tc.tile_pool(name="consts", bufs=1) as cpool, \
             tc.tile_pool(name="hpool", bufs=4) as hpool, \
             tc.tile_pool(name="dipool", bufs=2) as dipool, \
             tc.tile_pool(name="stpool", bufs=3) as spool, \
             tc.tile_pool(name="ypool", bufs=1, space="PSUM") as ypool, \
             tc.tile_pool(name="dpool", bufs=4, space="PSUM") as dpool:

            def load_const(dram, shape, dtype=f32, eng=None):
                t_ = cpool.tile(list(shape), dtype, name=dram.name + "_sb")
                (eng or nc.sync).dma_start(t_[:, :], dram[:, :])
                return t_

            w1x = load_const(w1x_d, (32, 128), f16)
            ib = load_const(ib_d, (4, 128), f16)
            x0t = load_const(x0t_d, (32, S), f16)
            i0b = load_const(i0b_d, (4, S), f16)
            wzz = load_const(wzz_d, (128, 128), f16, eng=nc.scalar)
            w1i = load_const(w1i_d, (8, 128), f16, eng=nc.scalar)
            w2d = load_const(w2d_d, (128, 32), f16, eng=nc.scalar)

            # y state: [parity][stream] -> [128, HALF] psum tile, all
            # initialized to y0 = x0 @ W1x + I0*w1i + b1 (fp32)
            ybank = [[ypool.tile([128, HALF], f32, name=f"y{p}{s}")
                      for s in range(2)] for p in range(2)]
            for p in range(2):
                for s in range(2):
                    nc.tensor.matmul(ybank[p][s][:, :], w1x[:, :],
                                     x0t[:, s * HALF:(s + 1) * HALF],
                                     start=True, stop=False,
                                     skip_group_check=True)
            for p in range(2):
                for s in range(2):
                    nc.tensor.matmul(ybank[p][s][:, :], ib[:, :],
                                     i0b[:, s * HALF:(s + 1) * HALF],
                                     start=False, stop=False,
                                     skip_group_check=True)

            di_tiles = {}

            def ensure_di(k, split=0):
                if k in di_tiles or k * GPF >= nmmi:
                    return
                g0 = k * GPF
                gsz = min(GPF, nmmi - g0)
                til = dipool.tile([8, GPF * S], f16, tag="di", name=f"di{k}")
                if split:
                    # fast head so step 0 isn't gated on the full group DMA
                    nc.gpsimd.dma_start(
                        til[:, :split * S].rearrange("p (g s) -> p g s", s=S),
                        di_d[:, g0:g0 + split, :],
                    )
                    nc.sync.dma_start(
                        til[:, split * S:gsz * S].rearrange(
                            "p (g s) -> p g s", s=S),
                        di_d[:, g0 + split:g0 + gsz, :],
                    )
                else:
                    nc.gpsimd.dma_start(
                        til[:, :gsz * S].rearrange("p (g s) -> p g s", s=S),
                        di_d[:, g0:g0 + gsz, :],
                    )
                di_tiles[k] = til

            ensure_di(0, split=10)
            ensure_di(1)

            h_pair = None
            prev_hA = prev_hB = None
            stage = None
            for t in range(nstep):
                e = t % 2
                u = t // 2
                if e == 0:
                    h_pair = hpool.tile([128, 2 * S], f16, tag="h", name=f"h{u}")
                if t % 8 == 0:
                    stage = spool.tile([128, 512], f32, tag="stage",
                                       name=f"st{t // 8}")
                    if nstep - t < 8:
                        # partial final chunk: zero-fill so the DMA below
                        # never reads unwritten SBUF
                        nc.any.memset(stage[:, :], 0.0)
                if t % GPF == 0 and t > 0:
                    ensure_di(t // GPF + 1)

                hA = h_pair[:, e * S:e * S + HALF]
                hB = h_pair[:, e * S + HALF:(e + 1) * S]
                yA, yB = ybank[e]
                last = t >= nstep - 1
                stop = t >= nstep - 3

                # ---- stream A slot ----
                nc.scalar.activation(hA, ybank[e][0][:, :], TANH)
                if not last:
                    zA, zB = ybank[1 - e]
                    k, s_ = divmod(t, GPF)
                    dA = di_tiles[k][:, s_ * S:s_ * S + HALF]
                    dB = di_tiles[k][:, s_ * S + HALF:(s_ + 1) * S]
                    # off-window updates: run on PE while tanh_A executes
                    nc.tensor.matmul(zA[:, :], w1i[:, :], dA,
                                     start=False, stop=False,
                                     skip_group_check=True)
                    nc.tensor.matmul(zB[:, :], w1i[:, :], dB,
                                     start=False, stop=False,
                                     skip_group_check=True)
                    if t >= 1:
                        nc.tensor.matmul(zA[:, :], wzz[:, :], prev_hA,
                                         start=False, stop=False,
                                         skip_group_check=True)
                    # window matmul: the only h_A-dependent y update
                    nc.tensor.matmul(zA[:, :], wzz[:, :], hA,
                                     start=False, stop=stop,
                                     skip_group_check=True)

                # ---- stream B slot ----
                nc.scalar.activation(hB, ybank[e][1][:, :], TANH)
                if not last:
                    if t >= 1:
                        nc.tensor.matmul(zB[:, :], wzz[:, :], prev_hB,
                                         start=False, stop=False,
                                         skip_group_check=True)
                    nc.tensor.matmul(zB[:, :], wzz[:, :], hB,
                                     start=False, stop=stop,
                                     skip_group_check=True)

                prev_hA, prev_hB = hA, hB

                if e == 1 or t == nstep - 1:
                    w = 2 * S if e == 1 else S
                    g = u % 4
                    dps = dpool.tile([32, 512], f32, tag="dps", name=f"dps{u}")
                    nc.tensor.matmul(dps[:, :w], w2d[:, :], h_pair[:, :w],
                                     start=True, stop=True,
                                     skip_group_check=True)
                    # partition-shifted evacuation packs 4 pairs into the
                    # 128-partition stage tile for a full-width DMA
                    nc.vector.tensor_copy(stage[32 * g:32 * g + 32, :w],
                                          dps[:, :w])

                if t % 8 == 7 or t == nstep - 1:
                    c = t // 8
                    nc.sync.dma_start(out_d[c, :, :], stage[:, :])
    nc.compile()
    return nc


def _host_prep(x0, current_profile, tgrid, W1, b1, W2, b2, nstep=NSTEP):
    """Build the shared constants and per-core inputs."""
    nmmi = nstep - 1
    dt = float(np.mean(np.diff(tgrid.astype(np.float64))))
    W1_64 = W1.astype(np.float64)
    W2_64 = W2.astype(np.float64)
    W1x = W1_64[:D]                      # [16, 64]
    w1iv = W1_64[D]                      # [64]
    M = dt * (W2_64 @ W1x)               # [64, 64]
    b2w = dt * (b2.astype(np.float64) @ W1x)   # [64]

    wzz = np.zeros((128, 128), np.float32)
    wzz[:64, :64] = M
    wzz[64:, 64:] = M
    w1i4 = np.zeros((4, 128), np.float32)
    w1i4[0, :64] = w1iv
    w1i4[1, :64] = b2w
    w1i4[2, 64:] = w1iv
    w1i4[3, 64:] = b2w
    w1i8 = np.concatenate([w1i4, w1i4], axis=0)     # [8, 128]
    w2d = np.zeros((128, 32), np.float32)
    w2d[:64, :16] = dt * W2_64
    w2d[64:, 16:] = dt * W2_64
    w1x_blk = np.zeros((32, 128), np.float32)
    w1x_blk[:16, :64] = W1x
    w1x_blk[16:, 64:] = W1x
    ib = np.zeros((4, 128), np.float32)
    ib[0, :64] = w1iv
    ib[1, :64] = b1
    ib[2, 64:] = w1iv
    ib[3, 64:] = b1
    shared = dict(wzz=wzz.astype(np.float16), w1i=w1i8.astype(np.float16),
                  w2d=w2d.astype(np.float16), w1x=w1x_blk.astype(np.float16),
                  ib=ib.astype(np.float16))

    in_maps = []
    for c in range(NCORES):
        xl = np.asarray(x0[c * BL:(c + 1) * BL], np.float32)     # [512, 16]
        Il = np.asarray(current_profile[c * BL:(c + 1) * BL], np.float32)
        x0t = np.zeros((32, S), np.float32)
        x0t[:16] = xl[:S].T
        x0t[16:] = xl[S:].T
        i0b = np.zeros((4, S), np.float32)
        i0b[0] = Il[:S, 0]
        i0b[1] = 1.0
        i0b[2] = Il[S:, 0]
        i0b[3] = 1.0
        dI = Il[:, 1:nmmi + 1] - Il[:, 0:nmmi]                   # [512, nmmi]
        di4 = np.zeros((max(nmmi, 1), 4, S), np.float32)
        if nmmi:
            di4[:, 0, :] = dI[:S].T
            di4[:, 1, :] = 1.0
            di4[:, 2, :] = dI[S:].T
            di4[:, 3, :] = 1.0
        # di8[t] applies both inc_{t-1}'s and inc_t's input terms: rows 0:4
        # are di4[t-1] (zeros at t=0), rows 4:8 are di4[t]
        di8 = np.zeros((max(nmmi, 1), 8, S), np.float32)
        if nmmi:
            di8[1:, 0:4] = di4[:-1]
            di8[:, 4:8] = di4
        # transpose to (8, nmmi, S) so each prefetch group is a contiguous
        # per-partition DMA slice
        di8t = np.ascontiguousarray(di8.transpose(1, 0, 2)).astype(np.float16)
        in_maps.append(dict(shared, x0t=x0t.astype(np.float16),
                        i0b=i0b.astype(np.float16), di=di8t))
    return dt, in_maps


def _host_decode(arr, xl, dt, b2, nstep=NSTEP):
    """arr: [nchunk, 128, 512] raw delta chunks for one core -> [BL, nstep+1, D]."""
    nchunk = (nstep + 7) // 8
    d6 = arr.reshape(nchunk, 4, 2, 16, 2, S)       # [c, g, q, d, e, s]
    d6 = d6.transpose(0, 1, 4, 2, 5, 3)            # [c, g, e, q, s, d]
    deltas = d6.reshape(nchunk * 8, BL, D)[:nstep].copy()
    deltas += (np.float32(dt) * b2)[None, None, :].astype(np.float32)
    xs = np.cumsum(deltas, axis=0, dtype=np.float32) + xl[None, :, :]
    out = np.empty((BL, nstep + 1, D), np.float32)
    out[:, 0] = xl
    out[:, 1:] = xs.transpose(1, 0, 2)
    return out


_NC_CACHE = {}


def _get_nc(nstep=NSTEP):
    if nstep not in _NC_CACHE:
        _NC_CACHE[nstep] = build_nc(nstep)
    return _NC_CACHE[nstep]


LAST_RESULTS = None


def kernel(x0, current_profile, t, W1, b1, W2, b2):
    global LAST_RESULTS
    x0 = np.asarray(x0, np.float32)
    current_profile = np.asarray(current_profile, np.float32)
    tgrid = np.asarray(t, np.float32)
    W1 = np.asarray(W1, np.float32)
    b1 = np.asarray(b1, np.float32)
    W2 = np.asarray(W2, np.float32)
    b2 = np.asarray(b2, np.float32)

    dt, in_maps = _host_prep(x0, current_profile, tgrid, W1, b1, W2, b2)
    nc = _get_nc()
    res = bass_utils.run_bass_kernel_spmd(
        nc, in_maps, core_ids=list(range(NCORES)),
        trace=bool(os.environ.get("KERNEL_TRACE")),
    )
    LAST_RESULTS = res

    out = np.empty((B, T, D), np.float32)
    for c in range(NCORES):
        xl = x0[c * BL:(c + 1) * BL]
        out[c * BL:(c + 1) * BL] = _host_decode(
            res.results[c]["delta"], xl, dt, b2)
    return out



# revision 2
# speedup vs baseline: 29.0065x; 29.0065x over previous
"""Trainium2 Bass kernel for the NeuralODE Euler-scan problem (v3).

Math reformulation (per core, local batch BL=512 split into 2 blocks of 256):
  reference: x_{t+1} = x_t + dt*(tanh([x_t, I_t] @ W1 + b1) @ W2 + b2)
  we track the pre-activation y_t = x_t @ W1x + I_t*w1i + b1 resident in PSUM:
      h_t     = tanh(y_t)                               (ACT, psum -> sbuf)
      y_{t+1} = y_t + h_t @ (dt*W2@W1x) + dI_t*w1i + dt*b2@W1x   (PE, accum)
  and — new in v3 — the solution x itself is accumulated ON THE PE:
      x_{t+1} = x_t + h_t @ (dt*W2)     (PE matmul, start=False into a
                                         persistent PSUM accumulator)
  so the T-cumsum runs in PSUM f32 for free; each step a DVE tensor_copy
  snapshots the accumulator to an f16 stage tile (the only per-step DVE
  work), and 16-step chunks are DMA'd out as f16.  The host decode is a
  pure layout transform + upcast — no cumsum.

Per-step engine budget (warm, errata cost model):
  ACT: 2 x tanh[128,128]          ~2*(222+128)/1.2  = 584 ns   <- pacing
  PE:  8 matmuls FD=128 + 5 LDW   ~8*56 + 5*30      = 598 ns
  DVE: 2 x copy[32,128] f32->f16  ~2*(120+128)/0.96 = 517 ns
  serial chains (tanh->wzz@h->tanh; w2d->snap->w2d) all < period.

The 256-sample free dim stays split into TWO staggered streams (A = cols
0:128, B = 128:256) with per-stream parity PSUM y banks exactly as v2:
PSUM dep tracking is bank-granular and each tile gets its own bank, so
the only cross-engine gate on the tanh chain is the single wzz@h matmul.
The x accumulators are one PSUM bank per stream; the per-step DVE
snapshot (WAR) serializes w2d(t+1) behind snap(t), a ~370ns cycle that
fits inside the ACT period.

All recurrence matmuls run in float16 (full PE rate at small moving dims;
10-bit mantissa keeps accumulated error ~2e-3, inside the 2e-2 gate).
f16 snapshots of x add < 5e-4 relative — the cumsum itself stays f32 in
PSUM.

Output: (nchunk, 32, 16*256) f16 = 8.4 MB/core (half of v2's f32 deltas),
decoded on host by transpose+astype only.  Batch dim (4096) sharded
across 8 cores; each runs this same program.

di prefetch DMA uses a host-transposed (8, nmmi, S) layout so each group
load is 8 contiguous descriptors instead of 240 strided ones.
"""

import os
import numpy as np

import concourse.bass as bass
from concourse import bacc
import concourse.mybir as mybir
from concourse.tile import TileContext
from concourse import bass_utils

B, T, D, H = 4096, 512, 16, 64
NCORES = 8
BL = B // NCORES          # 512 samples per core
S = BL // 2               # 256 samples per block
HALF = S // 2             # 128 samples per stream
NSTEP = T - 1             # 511 Euler steps
GPF = 30                  # dI prefetch group size (510 = 17*30)
KC = 16                   # steps per output chunk

f32 = mybir.dt.float32
f16 = mybir.dt.float16
TANH = mybir.ActivationFunctionType.Tanh


def build_nc(nstep=NSTEP, nchunk=None):
    nmmi = nstep - 1                # number of y-update steps (di8 rows)
    if nchunk is None:
        nchunk = (nstep + KC - 1) // KC
    nc = bacc.Bacc("TRN2", target_bir_lowering=False, debug=False)

    wzz_d = nc.dram_tensor("wzz", (128, 128), f16, kind="ExternalInput")
    w1i_d = nc.dram_tensor("w1i", (8, 128), f16, kind="ExternalInput")
    w2d_d = nc.dram_tensor("w2d", (128, 32), f16, kind="ExternalInput")
    w1x_d = nc.dram_tensor("w1x", (32, 128), f16, kind="ExternalInput")
    ib_d = nc.dram_tensor("ib", (4, 128), f16, kind="ExternalInput")
    id32_d = nc.dram_tensor("id32", (32, 32), f16, kind="ExternalInput")
    x0t_d = nc.dram_tensor("x0t", (32, S), f16, kind="ExternalInput")
    i0b_d = nc.dram_tensor("i0b", (4, S), f16, kind="ExternalInput")
    di_d = nc.dram_tensor("di", (8, max(nmmi, 1), S), f16, kind="ExternalInput")
    out_d = nc.dram_tensor("xout", (nchunk, 32, KC * S), f16,
                           kind="ExternalOutput")

    with TileContext(nc) as tc:
        with tc.tile_pool(name="consts", bufs=1) as cpool, \
             tc.tile_pool(name="hpool", bufs=4) as hpool, \
             tc.tile_pool(name="dipool", bufs=2) as dipool, \
             tc.tile_pool(name="stpool", bufs=3) as spool, \
             tc.tile_pool(name="ypool", bufs=1, space="PSUM") as ypool, \
             tc.tile_pool(name="xpool", bufs=1, space="PSUM") as xpool:

            def load_const(dram, shape, dtype=f32, eng=None):
                t_ = cpool.tile(list(shape), dtype, name=dram.name + "_sb")
                (eng or nc.sync).dma_start(t_[:, :], dram[:, :])
                return t_

            w1x = load_const(w1x_d, (32, 128), f16)
            ib = load_const(ib_d, (4, 128), f16)
            id32 = load_const(id32_d, (32, 32), f16)
            x0t = load_const(x0t_d, (32, S), f16)
            i0b = load_const(i0b_d, (4, S), f16)
            wzz = load_const(wzz_d, (128, 128), f16, eng=nc.scalar)
            w1i = load_const(w1i_d, (8, 128), f16, eng=nc.scalar)
            w2d = load_const(w2d_d, (128, 32), f16, eng=nc.scalar)

            # y state: [parity][stream] -> [128, HALF] psum tile, all
            # initialized to y0 = x0 @ W1x + I0*w1i + b1 (fp32)
            ybank = [[ypool.tile([128, HALF], f32, name=f"y{p}{s}")
                      for s in range(2)] for p in range(2)]
            for p in range(2):
                for s in range(2):
                    nc.tensor.matmul(ybank[p][s][:, :], w1x[:, :],
                                     x0t[:, s * HALF:(s + 1) * HALF],
                                     start=True, stop=False,
                                     skip_group_check=True)
            for p in range(2):
                for s in range(2):
                    nc.tensor.matmul(ybank[p][s][:, :], ib[:, :],
                                     i0b[:, s * HALF:(s + 1) * HALF],
                                     start=False, stop=False,
                                     skip_group_check=True)

            # x accumulators: [32, HALF] psum per stream, init to x0
            xacc = [xpool.tile([32, HALF], f32, name=f"x{s}")
                    for s in range(2)]
            for s in range(2):
                nc.tensor.matmul(xacc[s][:, :], id32[:, :],
                                 x0t[:, s * HALF:(s + 1) * HALF],
                                 start=True, stop=False,
                                 skip_group_check=True)

            di_tiles = {}

            def ensure_di(k, split=0):
                if k in di_tiles or k * GPF >= nmmi:
                    return
                g0 = k * GPF
                gsz = min(GPF, nmmi - g0)
                til = dipool.tile([8, GPF * S], f16, tag="di", name=f"di{k}")
                if split:
                    # fast head so step 0 isn't gated on the full group DMA
                    nc.gpsimd.dma_start(
                        til[:, :split * S].rearrange("p (g s) -> p g s", s=S),
                        di_d[:, g0:g0 + split, :],
                    )
                    nc.sync.dma_start(
                        til[:, split * S:gsz * S].rearrange(
                            "p (g s) -> p g s", s=S),
                        di_d[:, g0 + split:g0 + gsz, :],
                    )
                else:
                    nc.gpsimd.dma_start(
                        til[:, :gsz * S].rearrange("p (g s) -> p g s", s=S),
                        di_d[:, g0:g0 + gsz, :],
                    )
                di_tiles[k] = til

            ensure_di(0, split=10)
            ensure_di(1)

            prev_hA = prev_hB = None
            stage = None
            for t in range(nstep):
                e = t % 2
                u = t % KC
                if u == 0:
                    stage = spool.tile([32, KC * S], f16, tag="stage",
                                       name=f"st{t // KC}")
                    if nstep - t < KC:
                        # partial final chunk: zero-fill so the DMA below
                        # never reads unwritten SBUF
                        nc.any.memset(stage[:, :], 0.0)
                if t % GPF == 0 and t > 0:
                    ensure_di(t // GPF + 1)

                h = hpool.tile([128, S], f16, tag="h", name=f"h{t}")
                hA = h[:, :HALF]
                hB = h[:, HALF:]
                yA, yB = ybank[e]
                last = t >= nstep - 1
                stop = t >= nstep - 3

                # ---- stream A slot ----
                nc.scalar.activation(hA, yA[:, :], TANH)
                if not last:
                    zA, zB = ybank[1 - e]
                    k, s_ = divmod(t, GPF)
                    dA = di_tiles[k][:, s_ * S:s_ * S + HALF]
                    dB = di_tiles[k][:, s_ * S + HALF:(s_ + 1) * S]
                    # off-window updates: run on PE while tanh_A executes
                    nc.tensor.matmul(zA[:, :], w1i[:, :], dA,
                                     start=False, stop=False,
                                     skip_group_check=True)
                    nc.tensor.matmul(zB[:, :], w1i[:, :], dB,
                                     start=False, stop=False,
                                     skip_group_check=True)
                    if t >= 1:
                        nc.tensor.matmul(zA[:, :], wzz[:, :], prev_hA,
                                         start=False, stop=False,
                                         skip_group_check=True)
                        nc.tensor.matmul(zB[:, :], wzz[:, :], prev_hB,
                                         start=False, stop=False,
                                         skip_group_check=True)
                    # window matmul: the only h_A-dependent y update
                    nc.tensor.matmul(zA[:, :], wzz[:, :], hA,
                                     start=False, stop=stop,
                                     skip_group_check=True)
                # x_{t+1} += h_t @ (dt*W2): PSUM cumsum on the PE
                nc.tensor.matmul(xacc[0][:, :], w2d[:, :], hA,
                                 start=False, stop=last,
                                 skip_group_check=True)

                # ---- stream B slot ----
                nc.scalar.activation(hB, yB[:, :], TANH)
                if not last:
                    nc.tensor.matmul(zB[:, :], wzz[:, :], hB,
                                     start=False, stop=stop,
                                     skip_group_check=True)
                nc.tensor.matmul(xacc[1][:, :], w2d[:, :], hB,
                                 start=False, stop=last,
                                 skip_group_check=True)

                # f16 snapshots of x_{t+1} (DVE), packed into the stage tile
                nc.vector.tensor_copy(stage[:, u * S:u * S + HALF],
                                      xacc[0][:, :])
                nc.vector.tensor_copy(stage[:, u * S + HALF:(u + 1) * S],
                                      xacc[1][:, :])

                prev_hA, prev_hB = hA, hB

                if u == KC - 1 or last:
                    nc.sync.dma_start(out_d[t // KC, :, :], stage[:, :])
    nc.compile()
    return nc


def _host_prep(x0, current_profile, tgrid, W1, b1, W2, b2, nstep=NSTEP):
    """Build the shared constants and per-core inputs."""
    nmmi = nstep - 1
    dt = float(np.mean(np.diff(tgrid.astype(np.float64))))
    W1_64 = W1.astype(np.float64)
    W2_64 = W2.astype(np.float64)
    W1x = W1_64[:D]                      # [16, 64]
    w1iv = W1_64[D]                      # [64]
    M = dt * (W2_64 @ W1x)               # [64, 64]
    b2w = dt * (b2.astype(np.float64) @ W1x)   # [64]

    wzz = np.zeros((128, 128), np.float32)
    wzz[:64, :64] = M
    wzz[64:, 64:] = M
    w1i4 = np.zeros((4, 128), np.float32)
    w1i4[0, :64] = w1iv
    w1i4[1, :64] = b2w
    w1i4[2, 64:] = w1iv
    w1i4[3, 64:] = b2w
    w1i8 = np.concatenate([w1i4, w1i4], axis=0)     # [8, 128]
    w2d = np.zeros((128, 32), np.float32)
    w2d[:64, :16] = dt * W2_64
    w2d[64:, 16:] = dt * W2_64
    w1x_blk = np.zeros((32, 128), np.float32)
    w1x_blk[:16, :64] = W1x
    w1x_blk[16:, 64:] = W1x
    ib = np.zeros((4, 128), np.float32)
    ib[0, :64] = w1iv
    ib[1, :64] = b1
    ib[2, 64:] = w1iv
    ib[3, 64:] = b1
    id32 = np.eye(32, dtype=np.float32)
    shared = dict(wzz=wzz.astype(np.float16), w1i=w1i8.astype(np.float16),
                  w2d=w2d.astype(np.float16), w1x=w1x_blk.astype(np.float16),
                  ib=ib.astype(np.float16), id32=id32.astype(np.float16))

    in_maps = []
    for c in range(NCORES):
        xl = np.asarray(x0[c * BL:(c + 1) * BL], np.float32)     # [512, 16]
        Il = np.asarray(current_profile[c * BL:(c + 1) * BL], np.float32)
        x0t = np.zeros((32, S), np.float32)
        x0t[:16] = xl[:S].T
        x0t[16:] = xl[S:].T
        i0b = np.zeros((4, S), np.float32)
        i0b[0] = Il[:S, 0]
        i0b[1] = 1.0
        i0b[2] = Il[S:, 0]
        i0b[3] = 1.0
        dI = Il[:, 1:nmmi + 1] - Il[:, 0:nmmi]                   # [512, nmmi]
        di4 = np.zeros((max(nmmi, 1), 4, S), np.float32)
        if nmmi:
            di4[:, 0, :] = dI[:S].T
            di4[:, 1, :] = 1.0
            di4[:, 2, :] = dI[S:].T
            di4[:, 3, :] = 1.0
        # di8[t] applies both inc_{t-1}'s and inc_t's input terms: rows 0:4
        # are di4[t-1] (zeros at t=0), rows 4:8 are di4[t]
        di8 = np.zeros((max(nmmi, 1), 8, S), np.float32)
        if nmmi:
            di8[1:, 0:4] = di4[:-1]
            di8[:, 4:8] = di4
        # transpose to (8, nmmi, S) so each prefetch group is a contiguous
        # per-partition DMA slice
        di8t = np.ascontiguousarray(di8.transpose(1, 0, 2)).astype(np.float16)
        in_maps.append(dict(shared, x0t=x0t.astype(np.float16),
                        i0b=i0b.astype(np.float16), di=di8t))
    return dt, in_maps


def _host_decode(arr, xl, dt, b2, nstep=NSTEP):
    """arr: [nchunk, 32, KC*S] f16 x-snapshots for one core -> [BL, nstep+1, D]."""
    nchunk = (nstep + KC - 1) // KC
    a = arr.reshape(nchunk, 2, 16, KC, S)          # (c, b, d, u, s)
    a = a.transpose(1, 4, 0, 3, 2)                 # (b, s, c, u, d)
    xs = a.reshape(BL, nchunk * KC, D)[:, :nstep, :].astype(np.float32)
    if np.any(b2):
        corr = (np.arange(1, nstep + 1, dtype=np.float64)[:, None]
                * (dt * b2.astype(np.float64))[None, :]).astype(np.float32)
        xs = xs + corr[None, :, :]
    out = np.empty((BL, nstep + 1, D), np.float32)
    out[:, 0] = xl
    out[:, 1:] = xs
    return out


_NC_CACHE = {}


def _get_nc(nstep=NSTEP):
    if nstep not in _NC_CACHE:
        _NC_CACHE[nstep] = build_nc(nstep)
    return _NC_CACHE[nstep]


LAST_RESULTS = None


def kernel(x0, current_profile, t, W1, b1, W2, b2):
    global LAST_RESULTS
    x0 = np.asarray(x0, np.float32)
    current_profile = np.asarray(current_profile, np.float32)
    tgrid = np.asarray(t, np.float32)
    W1 = np.asarray(W1, np.float32)
    b1 = np.asarray(b1, np.float32)
    W2 = np.asarray(W2, np.float32)
    b2 = np.asarray(b2, np.float32)

    dt, in_maps = _host_prep(x0, current_profile, tgrid, W1, b1, W2, b2)
    nc = _get_nc()
    res = bass_utils.run_bass_kernel_spmd(
        nc, in_maps, core_ids=list(range(NCORES)),
        trace=bool(os.environ.get("KERNEL_TRACE")),
    )
    LAST_RESULTS = res

    out = np.empty((B, T, D), np.float32)
    for c in range(NCORES):
        xl = x0[c * BL:(c + 1) * BL]
        out[c * BL:(c + 1) * BL] = _host_decode(
            res.results[c]["xout"], xl, dt, b2)
    return out


# revision 3
# speedup vs baseline: 151.8851x; 5.2362x over previous
"""Trainium2 Bass kernel for the NeuralODE Euler-scan problem (v3).

Math reformulation (per core, local batch BL=512 split into 2 blocks of 256):
  reference: x_{t+1} = x_t + dt*(tanh([x_t, I_t] @ W1 + b1) @ W2 + b2)
  we track the pre-activation y_t = x_t @ W1x + I_t*w1i + b1 resident in PSUM:
      h_t     = tanh(y_t)                               (ACT, psum -> sbuf)
      y_{t+1} = y_t + h_t @ (dt*W2@W1x) + dI_t*w1i + dt*b2@W1x   (PE, accum)
  and — new in v3 — the solution x itself is accumulated ON THE PE:
      x_{t+1} = x_t + h_t @ (dt*W2)     (PE matmul, start=False into a
                                         persistent PSUM accumulator)
  so the T-cumsum runs in PSUM f32 for free; each step a DVE tensor_copy
  snapshots the accumulator to an f16 stage tile (the only per-step DVE
  work), and 16-step chunks are DMA'd out as f16.  The host decode is a
  pure layout transform + upcast — no cumsum.

Per-step engine budget (warm, errata cost model):
  ACT: 2 x tanh[128,128]          ~2*(222+128)/1.2  = 584 ns   <- pacing
  PE:  8 matmuls FD=128 + 5 LDW   ~8*56 + 5*30      = 598 ns
  DVE: 2 x copy[32,128] f32->f16  ~2*(120+128)/0.96 = 517 ns
  serial chains (tanh->wzz@h->tanh; w2d->snap->w2d) all < period.

The 256-sample free dim stays split into TWO staggered streams (A = cols
0:128, B = 128:256) with per-stream parity PSUM y banks exactly as v2:
PSUM dep tracking is bank-granular and each tile gets its own bank, so
the only cross-engine gate on the tanh chain is the single wzz@h matmul.
The x accumulators are one PSUM bank per stream; the per-step DVE
snapshot (WAR) serializes w2d(t+1) behind snap(t), a ~370ns cycle that
fits inside the ACT period.

All recurrence matmuls run in float16 (full PE rate at small moving dims;
10-bit mantissa keeps accumulated error ~2e-3, inside the 2e-2 gate).
f16 snapshots of x add < 5e-4 relative — the cumsum itself stays f32 in
PSUM.

Output: (nchunk, 32, 16*256) f16 = 8.4 MB/core (half of v2's f32 deltas),
decoded on host by transpose+astype only.  Batch dim (4096) sharded
across 8 cores; each runs this same program.

di prefetch DMA uses a host-transposed (8, nmmi, S) layout so each group
load is 8 contiguous descriptors instead of 240 strided ones.
"""

import os
import numpy as np

import concourse.bass as bass
from concourse import bacc
import concourse.mybir as mybir
from concourse.tile import TileContext
from concourse import bass_utils

B, T, D, H = 4096, 512, 16, 64
NCORES = 8
BL = B // NCORES          # 512 samples per core
S = BL // 2               # 256 samples per block
HALF = S // 2             # 128 samples per stream
NSTEP = T - 1             # 511 Euler steps
GPF = 30                  # dI prefetch group size (510 = 17*30)
KC = 16                   # steps per output chunk

f32 = mybir.dt.float32
f16 = mybir.dt.float16
TANH = mybir.ActivationFunctionType.Tanh


def build_nc(nstep=NSTEP, nchunk=None):
    nmmi = nstep - 1                # number of y-update steps (di8 rows)
    if nchunk is None:
        nchunk = (nstep + KC - 1) // KC
    nc = bacc.Bacc("TRN2", target_bir_lowering=False, debug=False)

    wzz_d = nc.dram_tensor("wzz", (128, 128), f16, kind="ExternalInput")
    w1i_d = nc.dram_tensor("w1i", (8, 128), f16, kind="ExternalInput")
    w2d_d = nc.dram_tensor("w2d", (128, 32), f16, kind="ExternalInput")
    w1x_d = nc.dram_tensor("w1x", (32, 128), f16, kind="ExternalInput")
    ib_d = nc.dram_tensor("ib", (4, 128), f16, kind="ExternalInput")
    id32_d = nc.dram_tensor("id32", (32, 32), f16, kind="ExternalInput")
    x0t_d = nc.dram_tensor("x0t", (32, S), f16, kind="ExternalInput")
    i0b_d = nc.dram_tensor("i0b", (4, S), f16, kind="ExternalInput")
    di_d = nc.dram_tensor("di", (8, max(nmmi, 1), S), f16, kind="ExternalInput")
    out_d = nc.dram_tensor("xout", (nchunk, 32, KC * S), f16,
                           kind="ExternalOutput")

    with TileContext(nc) as tc:
        with tc.tile_pool(name="consts", bufs=1) as cpool, \
             tc.tile_pool(name="hpool", bufs=4) as hpool, \
             tc.tile_pool(name="dipool", bufs=2) as dipool, \
             tc.tile_pool(name="stpool", bufs=3) as spool, \
             tc.tile_pool(name="ypool", bufs=1, space="PSUM") as ypool, \
             tc.tile_pool(name="xpool", bufs=1, space="PSUM") as xpool:

            def load_const(dram, shape, dtype=f32, eng=None):
                t_ = cpool.tile(list(shape), dtype, name=dram.name + "_sb")
                (eng or nc.sync).dma_start(t_[:, :], dram[:, :])
                return t_

            w1x = load_const(w1x_d, (32, 128), f16)
            ib = load_const(ib_d, (4, 128), f16)
            id32 = load_const(id32_d, (32, 32), f16)
            x0t = load_const(x0t_d, (32, S), f16)
            i0b = load_const(i0b_d, (4, S), f16)
            wzz = load_const(wzz_d, (128, 128), f16, eng=nc.scalar)
            w1i = load_const(w1i_d, (8, 128), f16, eng=nc.scalar)
            w2d = load_const(w2d_d, (128, 32), f16, eng=nc.scalar)

            # y state: [parity][stream] -> [128, HALF] psum tile, all
            # initialized to y0 = x0 @ W1x + I0*w1i + b1 (fp32)
            ybank = [[ypool.tile([128, HALF], f32, name=f"y{p}{s}")
                      for s in range(2)] for p in range(2)]
            for p in range(2):
                for s in range(2):
                    nc.tensor.matmul(ybank[p][s][:, :], w1x[:, :],
                                     x0t[:, s * HALF:(s + 1) * HALF],
                                     start=True, stop=False,
                                     skip_group_check=True)
            for p in range(2):
                for s in range(2):
                    nc.tensor.matmul(ybank[p][s][:, :], ib[:, :],
                                     i0b[:, s * HALF:(s + 1) * HALF],
                                     start=False, stop=False,
                                     skip_group_check=True)

            # x accumulators: [32, HALF] psum per stream, init to x0
            xacc = [xpool.tile([32, HALF], f32, name=f"x{s}")
                    for s in range(2)]
            for s in range(2):
                nc.tensor.matmul(xacc[s][:, :], id32[:, :],
                                 x0t[:, s * HALF:(s + 1) * HALF],
                                 start=True, stop=False,
                                 skip_group_check=True)

            di_tiles = {}

            def ensure_di(k, split=0):
                if k in di_tiles or k * GPF >= nmmi:
                    return
                g0 = k * GPF
                gsz = min(GPF, nmmi - g0)
                til = dipool.tile([8, GPF * S], f16, tag="di", name=f"di{k}")
                if split:
                    # fast head so step 0 isn't gated on the full group DMA
                    nc.gpsimd.dma_start(
                        til[:, :split * S].rearrange("p (g s) -> p g s", s=S),
                        di_d[:, g0:g0 + split, :],
                    )
                    nc.sync.dma_start(
                        til[:, split * S:gsz * S].rearrange(
                            "p (g s) -> p g s", s=S),
                        di_d[:, g0 + split:g0 + gsz, :],
                    )
                else:
                    nc.gpsimd.dma_start(
                        til[:, :gsz * S].rearrange("p (g s) -> p g s", s=S),
                        di_d[:, g0:g0 + gsz, :],
                    )
                di_tiles[k] = til

            ensure_di(0, split=10)
            ensure_di(1)

            prev_hA = prev_hB = None
            stage = None
            for t in range(nstep):
                e = t % 2
                u = t % KC
                if u == 0:
                    stage = spool.tile([32, KC * S], f16, tag="stage",
                                       name=f"st{t // KC}")
                    if nstep - t < KC:
                        # partial final chunk: zero-fill so the DMA below
                        # never reads unwritten SBUF
                        nc.any.memset(stage[:, :], 0.0)
                if t % GPF == 0 and t > 0:
                    ensure_di(t // GPF + 1)

                h = hpool.tile([128, S], f16, tag="h", name=f"h{t}")
                hA = h[:, :HALF]
                hB = h[:, HALF:]
                yA, yB = ybank[e]
                last = t >= nstep - 1
                stop = t >= nstep - 3

                # ---- stream A slot ----
                nc.scalar.activation(hA, yA[:, :], TANH)
                if not last:
                    zA, zB = ybank[1 - e]
                    k, s_ = divmod(t, GPF)
                    dA = di_tiles[k][:, s_ * S:s_ * S + HALF]
                    dB = di_tiles[k][:, s_ * S + HALF:(s_ + 1) * S]
                    # off-window updates: run on PE while tanh_A executes
                    nc.tensor.matmul(zA[:, :], w1i[:, :], dA,
                                     start=False, stop=False,
                                     skip_group_check=True)
                    nc.tensor.matmul(zB[:, :], w1i[:, :], dB,
                                     start=False, stop=False,
                                     skip_group_check=True)
                    if t >= 1:
                        nc.tensor.matmul(zA[:, :], wzz[:, :], prev_hA,
                                         start=False, stop=False,
                                         skip_group_check=True)
                        nc.tensor.matmul(zB[:, :], wzz[:, :], prev_hB,
                                         start=False, stop=False,
                                         skip_group_check=True)
                    # window matmul: the only h_A-dependent y update
                    nc.tensor.matmul(zA[:, :], wzz[:, :], hA,
                                     start=False, stop=stop,
                                     skip_group_check=True)
                # x_{t+1} += h_t @ (dt*W2): PSUM cumsum on the PE
                nc.tensor.matmul(xacc[0][:, :], w2d[:, :], hA,
                                 start=False, stop=last,
                                 skip_group_check=True)

                # ---- stream B slot ----
                nc.scalar.activation(hB, yB[:, :], TANH)
                if not last:
                    nc.tensor.matmul(zB[:, :], wzz[:, :], hB,
                                     start=False, stop=stop,
                                     skip_group_check=True)
                nc.tensor.matmul(xacc[1][:, :], w2d[:, :], hB,
                                 start=False, stop=last,
                                 skip_group_check=True)

                # f16 snapshots of x_{t+1} (DVE), packed into the stage tile
                nc.vector.tensor_copy(stage[:, u * S:u * S + HALF],
                                      xacc[0][:, :])
                nc.vector.tensor_copy(stage[:, u * S + HALF:(u + 1) * S],
                                      xacc[1][:, :])

                prev_hA, prev_hB = hA, hB

                if u == KC - 1 or last:
                    nc.sync.dma_start(out_d[t // KC, :, :], stage[:, :])
    nc.compile()
    return nc


def _host_prep(x0, current_profile, tgrid, W1, b1, W2, b2, nstep=NSTEP):
    """Build the shared constants and per-core inputs."""
    nmmi = nstep - 1
    dt = float(np.mean(np.diff(tgrid.astype(np.float64))))
    W1_64 = W1.astype(np.float64)
    W2_64 = W2.astype(np.float64)
    W1x = W1_64[:D]                      # [16, 64]
    w1iv = W1_64[D]                      # [64]
    M = dt * (W2_64 @ W1x)               # [64, 64]
    b2w = dt * (b2.astype(np.float64) @ W1x)   # [64]

    wzz = np.zeros((128, 128), np.float32)
    wzz[:64, :64] = M
    wzz[64:, 64:] = M
    w1i4 = np.zeros((4, 128), np.float32)
    w1i4[0, :64] = w1iv
    w1i4[1, :64] = b2w
    w1i4[2, 64:] = w1iv
    w1i4[3, 64:] = b2w
    w1i8 = np.concatenate([w1i4, w1i4], axis=0)     # [8, 128]
    w2d = np.zeros((128, 32), np.float32)
    w2d[:64, :16] = dt * W2_64
    w2d[64:, 16:] = dt * W2_64
    w1x_blk = np.zeros((32, 128), np.float32)
    w1x_blk[:16, :64] = W1x
    w1x_blk[16:, 64:] = W1x
    ib = np.zeros((4, 128), np.float32)
    ib[0, :64] = w1iv
    ib[1, :64] = b1
    ib[2, 64:] = w1iv
    ib[3, 64:] = b1
    id32 = np.eye(32, dtype=np.float32)
    shared = dict(wzz=wzz.astype(np.float16), w1i=w1i8.astype(np.float16),
                  w2d=w2d.astype(np.float16), w1x=w1x_blk.astype(np.float16),
                  ib=ib.astype(np.float16), id32=id32.astype(np.float16))

    in_maps = []
    for c in range(NCORES):
        xl = np.asarray(x0[c * BL:(c + 1) * BL], np.float32)     # [512, 16]
        Il = np.asarray(current_profile[c * BL:(c + 1) * BL], np.float32)
        x0t = np.zeros((32, S), np.float32)
        x0t[:16] = xl[:S].T
        x0t[16:] = xl[S:].T
        i0b = np.zeros((4, S), np.float32)
        i0b[0] = Il[:S, 0]
        i0b[1] = 1.0
        i0b[2] = Il[S:, 0]
        i0b[3] = 1.0
        dI = Il[:, 1:nmmi + 1] - Il[:, 0:nmmi]                   # [512, nmmi]
        di4 = np.zeros((max(nmmi, 1), 4, S), np.float32)
        if nmmi:
            di4[:, 0, :] = dI[:S].T
            di4[:, 1, :] = 1.0
            di4[:, 2, :] = dI[S:].T
            di4[:, 3, :] = 1.0
        # di8[t] applies both inc_{t-1}'s and inc_t's input terms: rows 0:4
        # are di4[t-1] (zeros at t=0), rows 4:8 are di4[t]
        di8 = np.zeros((max(nmmi, 1), 8, S), np.float32)
        if nmmi:
            di8[1:, 0:4] = di4[:-1]
            di8[:, 4:8] = di4
        # transpose to (8, nmmi, S) so each prefetch group is a contiguous
        # per-partition DMA slice
        di8t = np.ascontiguousarray(di8.transpose(1, 0, 2)).astype(np.float16)
        in_maps.append(dict(shared, x0t=x0t.astype(np.float16),
                        i0b=i0b.astype(np.float16), di=di8t))
    return dt, in_maps


def _host_decode(arr, xl, dt, b2, nstep=NSTEP):
    """arr: [nchunk, 32, KC*S] f16 x-snapshots for one core -> [BL, nstep+1, D]."""
    nchunk = (nstep + KC - 1) // KC
    a = arr.reshape(nchunk, 2, 16, KC, S)          # (c, b, d, u, s)
    a = a.transpose(1, 4, 0, 3, 2)                 # (b, s, c, u, d)
    # strided astype does the gather + upcast in one pass
    xs = a.astype(np.float32).reshape(BL, nchunk * KC, D)[:, :nstep, :]
    if np.any(b2):
        corr = (np.arange(1, nstep + 1, dtype=np.float64)[:, None]
                * (dt * b2.astype(np.float64))[None, :]).astype(np.float32)
        xs = xs + corr[None, :, :]
    out = np.empty((BL, nstep + 1, D), np.float32)
    out[:, 0] = xl
    out[:, 1:] = xs
    return out


_NC_CACHE = {}


def _get_nc(nstep=NSTEP):
    if nstep not in _NC_CACHE:
        _NC_CACHE[nstep] = build_nc(nstep)
    return _NC_CACHE[nstep]


LAST_RESULTS = None


def kernel(x0, current_profile, t, W1, b1, W2, b2):
    global LAST_RESULTS
    x0 = np.asarray(x0, np.float32)
    current_profile = np.asarray(current_profile, np.float32)
    tgrid = np.asarray(t, np.float32)
    W1 = np.asarray(W1, np.float32)
    b1 = np.asarray(b1, np.float32)
    W2 = np.asarray(W2, np.float32)
    b2 = np.asarray(b2, np.float32)

    dt, in_maps = _host_prep(x0, current_profile, tgrid, W1, b1, W2, b2)
    nc = _get_nc()
    res = bass_utils.run_bass_kernel_spmd(
        nc, in_maps, core_ids=list(range(NCORES)),
        trace=bool(os.environ.get("KERNEL_TRACE")),
    )
    LAST_RESULTS = res

    out = np.empty((B, T, D), np.float32)
    for c in range(NCORES):
        xl = x0[c * BL:(c + 1) * BL]
        out[c * BL:(c + 1) * BL] = _host_decode(
            res.results[c]["xout"], xl, dt, b2)
    return out
